# revision 20
# baseline (speedup 1.0000x reference)
"""CDGM (graph-construction GNN) fused kernel for Trainium2, 8-way row-sharded.

Math per layer (reference):
    gl   = relu(x @ Wgl + bgl)                      [N, F]
    t_ij = ||gl_i - gl_j||^2  (via sq_i + sq_j - 2 gl_i.gl_j)
    adj  = sigmoid(-(1+temp)*sqrt(relu(t)+eps) * (t>0) + (5+theta))
    x    = (adj @ (x @ Wgnn + bgnn)) / rowsum(adj)   (+relu except last layer)
    out  = softmax(x)

Device strategy (per core r, query rows Q_r = [1024*r, 1024*(r+1))):
  - glTs = sqrt(2)*gl stored transposed [F, N] fp16; t' = -t accumulated in
    PSUM f32 as (glTs_j . glTs_q) - sq_j - sq_q (K=F fp16 matmul + K=2
    rank-2 correction for layer 0; layer 1 folds both rank-1 terms into the
    main matmul, K=66). sq comes from the same fp16-quantized glTs so the
    cancellation is consistent.
  - ACT reads the PSUM t' tiles directly: sqrt(-1*t' + DELTA) -> fp16 slab.
    The free affine (scale=-1, bias=DELTA) replaces the old DVE clamp pass;
    DELTA=0.02 keeps the sqrt domain non-negative (max positive t'
    excursion measured 0.0073) at ~1e-3 adjacency error.
  - ACT phases are strictly batched per layer (all sqrt, then all sigmoid)
    so the table set switches exactly 4 times total.
  - adj@h runs transposed on both layers: yT[f, q] += h_jc^T @ slab_jc
    (h chunks are the stationary operand). Layer 0 computes deg with
    separate ones-weight M=1 matmuls, divides via DVE (reciprocal + PE
    partition-broadcast + scalar_tensor_tensor) and writes x1T directly in
    the transposed layout the next layer wants - no PE transposes.
  - x1T is all-gathered in two 512-column chunks so the first collective
    overlaps the second half of layer-0's sigmoid/adj@h work. Layer 1
    consumes gathered columns evens-first so its setup can start as soon
    as the first chunk lands.
  - SBUF liveness: the two 32K-wide slabs (128 KB/partition) are global;
    layer-0 streams xT through double-buffered 2048-col blocks and scopes
    glTs/aug tiles in a pool that closes after the t' loop, so layer-1's
    setup allocates into space whose last readers finished by mid-layer-0
    (keeps the layer boundary pipelined). h_nat0 is global because its
    last reader is layer-0's final y matmul.
  - Final division + softmax run on host (y and deg returned raw in f32).

The harness calls kernel(**inputs) with full inputs; sharding is internal.
"""

import math

import numpy as np

N = 8192
D_IN = 256
F0, F1 = 128, 64
N_CORES = 8
QR = N // N_CORES          # query rows per core
NJ = N // 128              # 64 j-chunks of 128
SLABW = NJ * 512           # slab free width per q-tile (32768)
DELTA = 0.02               # sqrt domain guard (see module docstring)

_CACHE = {}


def _build(temp: float, theta: float):
    import concourse.bacc as bacc
    import concourse.mybir as mybir
    import concourse.tile as tile
    from concourse.tile_rust import add_dep_helper
    from contextlib import ExitStack

    DT = mybir.dt
    AF = mybir.ActivationFunctionType
    ALU = mybir.AluOpType
    F32, FP16 = DT.float32, DT.float16

    sig_scale = -(1.0 + temp)
    sig_bias = 5.0 + theta

    nc = bacc.Bacc(
        "TRN2", target_bir_lowering=False, debug=False, enable_asserts=False,
        num_devices=N_CORES,
    )

    # ---- I/O ----
    xTf_in = nc.dram_tensor("xTf", [D_IN, N], FP16, kind="ExternalInput").ap()
    xTq_in = nc.dram_tensor("xTq", [D_IN, QR], FP16, kind="ExternalInput").ap()
    wglx_in = [
        nc.dram_tensor("wglx0", [D_IN, F0], FP16, kind="ExternalInput").ap(),
        nc.dram_tensor("wglx1", [F0, F1], FP16, kind="ExternalInput").ap(),
    ]
    bglx_in = [
        nc.dram_tensor("bglx0", [F0, 1], F32, kind="ExternalInput").ap(),
        nc.dram_tensor("bglx1", [F1, 1], F32, kind="ExternalInput").ap(),
    ]
    wgna_in = [
        nc.dram_tensor("wgna0", [D_IN, F0], FP16, kind="ExternalInput").ap(),
        nc.dram_tensor("wgna1", [F0, F1], FP16, kind="ExternalInput").ap(),
    ]
    bgn0_in = nc.dram_tensor("bgn0", [F0, 1], F32, kind="ExternalInput").ap()
    y_out = nc.dram_tensor("y_out", [F1 + 1, QR], F32, kind="ExternalOutput").ap()

    with tile.TileContext(nc) as tc, ExitStack() as ctx:
        pconst = ctx.enter_context(tc.tile_pool(name="const", bufs=1))
        pouter = ctx.enter_context(tc.tile_pool(name="outer", bufs=1))
        pdram = ctx.enter_context(tc.tile_pool(name="dram", bufs=1, space="DRAM"))
        psA = ctx.enter_context(tc.tile_pool(name="psA", bufs=2, space="PSUM"))
        psB = ctx.enter_context(tc.tile_pool(name="psB", bufs=2, space="PSUM"))

        # ---- constants ----
        neghalf = pconst.tile([128, 1], FP16, tag="neghalf")
        nc.gpsimd.memset(neghalf[:], -0.5)
        ones_l = pconst.tile([128, 1], FP16, tag="ones_l")   # deg weights
        nc.gpsimd.memset(ones_l[:], 1.0)
        ones_r = pconst.tile([1, 128], FP16, tag="ones_r")   # recip broadcast
        nc.gpsimd.memset(ones_r[:], 1.0)
        sgb = pconst.tile([128, 1], F32, tag="sgb")
        nc.gpsimd.memset(sgb[:], sig_bias)
        sgs = pconst.tile([128, 1], F32, tag="sgs")
        nc.gpsimd.memset(sgs[:], sig_scale)
        delta_t = pconst.tile([128, 1], F32, tag="delta_t")
        nc.gpsimd.memset(delta_t[:], DELTA)

        fins = [D_IN, F0]
        fouts = [F0, F1]
        wgl = []
        bgl = []
        wgna = []
        for li in range(2):
            fin, fout = fins[li], fouts[li]
            nk = fin // 128
            wk = []
            ak = []
            for k in range(nk):
                t = pconst.tile([128, fout], FP16, tag=f"wgl{li}_{k}")
                nc.sync.dma_start(t[:], wglx_in[li][k * 128:(k + 1) * 128, :])
                wk.append(t)
                t = pconst.tile([128, fout], FP16, tag=f"wgna{li}_{k}")
                nc.sync.dma_start(t[:], wgna_in[li][k * 128:(k + 1) * 128, :])
                ak.append(t)
            wgl.append(wk)
            wgna.append(ak)
            bt = pconst.tile([fout, 1], F32, tag=f"bgl{li}")
            nc.sync.dma_start(bt[:], bglx_in[li][:])
            bgl.append(bt)
        bgn0 = pconst.tile([F0, 1], F32, tag="bgn0")
        nc.sync.dma_start(bgn0[:], bgn0_in[:])
        # high-half bias staged at base partition 0: the y_b accumulator
        # (features 64:127) lives on PSUM partitions 0:63, and DVE lanes
        # cannot cross partitions - the shift happens in the final DMA
        bgn0b = pconst.tile([64, 1], F32, tag="bgn0b")
        nc.sync.dma_start(bgn0b[:], bgn0_in[64:128, :])

        # layer-0 -> layer-1 bridge (x1 transposed, this core's columns),
        # all-gathered in two 512-column chunks
        x1Tq = pouter.tile([F0, QR], FP16, tag="x1tq")
        aginH = [pdram.tile([F0, 512], FP16, tag=f"agin{h}", name=f"agin{h}")
                 for h in (0, 1)]
        agoutH = [pdram.tile([N_CORES * F0, 512], FP16, tag=f"agout{h}",
                             name=f"agout{h}") for h in (0, 1)]

        # global: the two q-half slabs and layer-0's h (its last reader is
        # the very end of layer 0, so it must never alias layer-1 tiles)
        slabs = [pouter.tile([128, SLABW], FP16, tag=f"slab{qt}", name=f"slab{qt}")
                 for qt in (0, 1)]
        h_nat0 = pouter.tile([128, NJ * (F0 + 2)], FP16, tag="hnat0")

        prev_sig = None
        for li in range(2):
            fin, fout = fins[li], fouts[li]
            nk = fin // 128
            # layer 1 fuses both -sq rank-1 terms into the main matmul as two
            # extra contraction rows (K = 66: [gl; -sqT; 1] x [gl; 1; -sqTq]).
            # Layer 0 (fout=128) cannot, and uses a separate K=2 correction.
            fka = fout + 2 if li == 1 else fout
            # h chunk layout: layer 0 [h(0:64) | ones | h(64:128) | ones]
            # (130 wide, so adj@h splits into two M=65 stationary tiles -
            # uniform M, because alternating matmul tile sizes inside open
            # PSUM accumulation groups serializes the PE pipeline at +120ns
            # per matmul; the first ones column gives deg in row 64 of y_a,
            # the second is a dummy). Layer 1: [h | ones] (65 wide).
            hfp = fout + 2 if li == 0 else fout + 1
            # layer 1's gathered x1T lands in two column-interleaved halves;
            # process 512-col chunks (and the j-chunks inside them) in
            # half-0-first order so work starts before the second AllGather.
            if li == 0:
                corder = list(range(16))
            else:
                corder = [c for c in range(16) if c % 2 == 0] + \
                         [c for c in range(16) if c % 2 == 1]
            jcs = [jc for c in corder for jc in range(4 * c, 4 * c + 4)]
            with ExitStack() as lctx:
                pmain = lctx.enter_context(tc.tile_pool(name=f"main{li}", bufs=1))
                if li == 0:
                    h_nat = h_nat0
                else:
                    h_nat = pmain.tile([128, NJ * hfp], FP16, tag="hnat")
                h3 = h_nat.rearrange("p (n f) -> p n f", f=hfp)
                if li == 0:
                    nc.gpsimd.memset(h3[:, :, 64:65], 1.0)
                    nc.gpsimd.memset(h3[:, :, 129:130], 1.0)
                else:
                    nc.gpsimd.memset(h3[:, :, fout:fout + 1], 1.0)

                with ExitStack() as tctx:
                    # closes right after the t' loop: everything here is
                    # dead once the last t' matmul has run
                    ptp = tctx.enter_context(tc.tile_pool(name=f"tp{li}", bufs=1))
                    glTs = ptp.tile([128, N], FP16, tag="glTs")
                    glTsq = ptp.tile([128, QR], FP16, tag="glTsq")
                    sqqstage = ptp.tile([1, QR], FP16, tag="sqqstage")
                    # K=128 everywhere: matmuls with K in (64,128) run at
                    # 427ns vs 216ns for K=128 (measured), so the rank-2
                    # correction tensors are zero-padded to full 128 rows.
                    if li == 0:
                        # augL: row0 = -sqT, row64 = ones, rest 0
                        # augQ: row0 = ones, row64 = -sqTq, rest 0
                        augL = ptp.tile([128, N], FP16, tag="augL")
                        augQ = ptp.tile([128, QR], FP16, tag="augQ")
                        nc.gpsimd.memset(augL[:], 0.0)
                        nc.gpsimd.memset(augL[64:65, :], 1.0)
                        nc.gpsimd.memset(augQ[:], 0.0)
                        nc.gpsimd.memset(augQ[0:1, :], 1.0)
                    else:
                        # glTs: rows 0:64 gl, row64 = ones, row65 = -sqT,
                        # rest 0; glTsq: rows 0:64 gl, row64 = -sqTq,
                        # row65 = ones, rest 0. Engines cannot start at
                        # partition 65/66, so row 65 is filled by DMA and
                        # the zero/ones fills start at partition 64.
                        nc.gpsimd.memset(glTs[64:128, :], 0.0)
                        nc.gpsimd.memset(glTs[64:65, :], 1.0)
                        nc.gpsimd.memset(glTsq[64:128, :], 0.0)
                        onesq = ptp.tile([1, QR], FP16, tag="onesq")
                        nc.gpsimd.memset(onesq[:], 1.0)
                        nc.sync.dma_start(glTsq[65:66, :], onesq[:])

                    def cstage(c, pg):
                        """evict glTs chunk c from PSUM + its -sq entries"""
                        nc.vector.tensor_scalar(
                            glTs[0:fout, c * 512:(c + 1) * 512], pg[:],
                            bgl[li][:], 0.0, ALU.add, ALU.max,
                        )
                        gl2c = pmain.tile([fout, 512], FP16, tag="gl2c")
                        nc.vector.tensor_mul(
                            gl2c[:], glTs[0:fout, c * 512:(c + 1) * 512],
                            glTs[0:fout, c * 512:(c + 1) * 512],
                        )
                        pq = psB.tile([1, 512], F32, tag="oacc")
                        nc.tensor.matmul(pq[:], neghalf[0:fout, :], gl2c[:],
                                         start=True, stop=True)
                        if li == 0:
                            nc.vector.tensor_copy(
                                augL[0:1, c * 512:(c + 1) * 512], pq[:])
                        else:
                            sqj = pmain.tile([1, 512], FP16, tag="sqj", bufs=2)
                            nc.vector.tensor_copy(sqj[:], pq[:])
                            nc.sync.dma_start(
                                glTs[65:66, c * 512:(c + 1) * 512], sqj[:])

                    # ======== setup ========
                    with ExitStack() as sctx:
                        pxt = sctx.enter_context(
                            tc.tile_pool(name=f"xt{li}", bufs=1))
                        # q side first so the first t' group unblocks early
                        if li == 0:
                            xtq = [pxt.tile([128, QR], FP16, tag=f"xtq{k}",
                                            name=f"xtq{k}") for k in range(nk)]
                            for k in range(nk):
                                nc.sync.dma_start(
                                    xtq[k][:], xTq_in[k * 128:(k + 1) * 128, :])
                        else:
                            xtq = [x1Tq]
                        for c in range(QR // 512):
                            pg = psB.tile([fout, 512], F32, tag="oacc")
                            for k in range(nk):
                                nc.tensor.matmul(
                                    pg[:], wgl[li][k][:],
                                    xtq[k][:, c * 512:(c + 1) * 512],
                                    start=(k == 0), stop=(k == nk - 1),
                                )
                            nc.vector.tensor_scalar(
                                glTsq[0:fout, c * 512:(c + 1) * 512], pg[:],
                                bgl[li][:], 0.0, ALU.add, ALU.max,
                            )
                            gl2c = pmain.tile([fout, 512], FP16, tag="gl2c")
                            nc.vector.tensor_mul(
                                gl2c[:], glTsq[0:fout, c * 512:(c + 1) * 512],
                                glTsq[0:fout, c * 512:(c + 1) * 512],
                            )
                            pq = psB.tile([1, 512], F32, tag="oacc")
                            nc.tensor.matmul(pq[:], neghalf[0:fout, :], gl2c[:],
                                             start=True, stop=True)
                            nc.vector.tensor_copy(
                                sqqstage[0:1, c * 512:(c + 1) * 512], pq[:])
                        if li == 0:
                            nc.sync.dma_start(augQ[64:65, :], sqqstage[:])
                        else:
                            nc.sync.dma_start(glTsq[64:65, :], sqqstage[:])

                        if li == 0:
                            # stream xT through double-buffered 1024-col
                            # blocks; build glTs, -sq and h_nat per block
                            for b in range(8):
                                xb = [pxt.tile([128, 1024], FP16, tag=f"xtf{k}",
                                               name=f"xtf{k}", bufs=3)
                                      for k in range(nk)]
                                for k in range(nk):
                                    nc.sync.dma_start(
                                        xb[k][:],
                                        xTf_in[k * 128:(k + 1) * 128,
                                               b * 1024:(b + 1) * 1024],
                                    )
                                for c4 in range(2):
                                    c = 2 * b + c4
                                    pg = psB.tile([fout, 512], F32, tag="oacc")
                                    for k in range(nk):
                                        nc.tensor.matmul(
                                            pg[:], wgl[li][k][:],
                                            xb[k][:, c4 * 512:(c4 + 1) * 512],
                                            start=(k == 0), stop=(k == nk - 1),
                                        )
                                    cstage(c, pg)
                                for g4 in range(2):
                                    j0 = 8 * b + 4 * g4
                                    ph = psB.tile([128, 4 * fout], F32, tag="oacc")
                                    for t in range(4):
                                        jl = 4 * g4 + t
                                        sl = ph[:, t * fout:(t + 1) * fout]
                                        for k in range(nk):
                                            nc.tensor.matmul(
                                                sl, xb[k][:, jl * 128:(jl + 1) * 128],
                                                wgna[li][k][:],
                                                start=(k == 0), stop=(k == nk - 1),
                                            )
                                    ph3 = ph[:].rearrange(
                                        "p (n f) -> p n f", f=fout)
                                    nc.vector.tensor_copy(
                                        h3[:, j0:j0 + 4, 0:64], ph3[:, :, 0:64])
                                    nc.vector.tensor_copy(
                                        h3[:, j0:j0 + 4, 65:129],
                                        ph3[:, :, 64:128])
                        else:
                            # gathered x1T, consumed evens-first
                            xtf = pxt.tile([128, N], FP16, tag="xtf1f",
                                           name="xtf1f")
                            for h in (0, 1):
                                for r in range(N_CORES):
                                    nc.sync.dma_start(
                                        xtf[:, r * QR + h * 512:
                                            r * QR + (h + 1) * 512],
                                        agoutH[h][r * F0:(r + 1) * F0, :],
                                    )
                            for c in corder:
                                pg = psB.tile([fout, 512], F32, tag="oacc")
                                nc.tensor.matmul(
                                    pg[:], wgl[li][0][:],
                                    xtf[:, c * 512:(c + 1) * 512],
                                    start=True, stop=True,
                                )
                                cstage(c, pg)
                            for g0 in range(0, NJ, 4):
                                js = jcs[g0:g0 + 4]
                                ph = psB.tile([128, 4 * fout], F32, tag="oacc")
                                for t in range(4):
                                    n_ = js[t]
                                    nc.tensor.matmul(
                                        ph[:, t * fout:(t + 1) * fout],
                                        xtf[:, n_ * 128:(n_ + 1) * 128],
                                        wgna[li][0][:],
                                        start=True, stop=True,
                                    )
                                nc.vector.tensor_copy(
                                    h3[:, js[0]:js[0] + 4, 0:fout],
                                    ph[:].rearrange("p (n f) -> p n f", f=fout),
                                )

                    # ======== t' + sqrt (ACT consumes PSUM directly) ========
                    # qt0/qt1 interleaved: consecutive matmuls then stream
                    # DIFFERENT rhs slices, which keeps the LDWEIGHTS
                    # pull-ahead working (a repeated rhs with changing
                    # weights measures 334-427 ns/MM vs 216 warm)
                    sqrt_insts = []
                    for g0 in range(0, NJ, 2):
                        tms = [psA.tile([128, 1024], F32, tag="tmac",
                                        name=f"tm{qt}") for qt in (0, 1)]
                        for t in range(2):
                            j = jcs[g0 + t]
                            for qt in (0, 1):
                                nc.tensor.matmul(
                                    tms[qt][:, t * 512:(t + 1) * 512],
                                    glTs[:, j * 128:(j + 1) * 128],
                                    glTsq[:, qt * 512:(qt + 1) * 512],
                                    start=True, stop=(li == 1),
                                )
                        if li == 0:
                            for t in range(2):
                                j = jcs[g0 + t]
                                for qt in (0, 1):
                                    nc.tensor.matmul(
                                        tms[qt][:, t * 512:(t + 1) * 512],
                                        augL[:, j * 128:(j + 1) * 128],
                                        augQ[:, qt * 512:(qt + 1) * 512],
                                        start=False, stop=True,
                                    )
                        for qt in (0, 1):
                            si = nc.scalar.activation(
                                slabs[qt][:, g0 * 512:(g0 + 2) * 512],
                                tms[qt][:, 0:1024],
                                AF.Sqrt, bias=delta_t[:], scale=-1.0,
                            )
                            if prev_sig is not None:
                                add_dep_helper(si.ins, prev_sig.ins, sync=False,
                                               reason="act-table phase batching")
                            sqrt_insts.append(si)

                # ======== sigmoid + adj@h + bridge ========
                sig_insts = []
                for qt in (0, 1):
                    slab = slabs[qt]
                    for g in range(4):
                        sl = slab[:, g * 8192:(g + 1) * 8192]
                        si = nc.scalar.activation(sl, sl, AF.Sigmoid,
                                                  bias=sgb[:], scale=sgs[:])
                        # Keep all sigmoids after every sqrt so the ACT
                        # table set switches exactly once per phase.
                        add_dep_helper(si.ins, sqrt_insts[-1].ins, sync=False,
                                       reason="act-table phase batching")
                        sig_insts.append(si)
                prev_sig = sig_insts[-1]

                # created here (not at layer start) so its space is carved out
                # of what the tp pool freed, after the t' loop
                pev = lctx.enter_context(tc.tile_pool(name=f"ev{li}", bufs=1))
                wt = psB.tile([128, 512], F32, tag="oacc", name="warm")
                for w in range(16):
                    mi = nc.tensor.matmul(wt[:], wgna[0][0][:],
                                          h_nat0[:, 0:512], start=True,
                                          stop=True)
                    if w == 0:
                        add_dep_helper(mi.ins, sig_insts[0].ins, sync=False,
                                       reason="HAM warm-up gate")
                for qt in (0, 1):
                    slab = slabs[qt]
                    if li == 0:
                        # adj@[h|1] split into M=65 / M=64 halves: two
                        # accumulators alternate, so consecutive matmuls never
                        # accumulate into the same PSUM region (which would
                        # serialize on the PE drain); deg rides along as
                        # row 64 of y_a via the ones column of h.
                        y_a = psA.tile([65, 512], F32, tag="acc", name="y_a")
                        y_b = psA.tile([65, 512], F32, tag="acc", name="y_b")
                        # staggered by one slab slice so consecutive
                        # matmuls stream different rhs regions (see t' note)
                        for i in range(NJ + 1):
                            if i < NJ:
                                j = jcs[i]
                                nc.tensor.matmul(
                                    y_a[:], h_nat[:, j * hfp:j * hfp + 65],
                                    slab[:, i * 512:(i + 1) * 512],
                                    start=(i == 0), stop=(i == NJ - 1),
                                )
                            if i > 0:
                                j = jcs[i - 1]
                                nc.tensor.matmul(
                                    y_b[:],
                                    h_nat[:, j * hfp + 65:(j + 1) * hfp],
                                    slab[:, (i - 1) * 512:i * 512],
                                    start=(i == 1), stop=(i == NJ),
                                )
                        degS = pev.tile([1, 512], F32, tag="degS")
                        nc.vector.tensor_copy(degS[:], y_a[64:65, :])
                        recipF = pev.tile([1, 512], F32, tag="recipF")
                        nc.vector.reciprocal_approx_fast(recipF[:], degS[:])
                        recipH = pev.tile([1, 512], FP16, tag="recipH")
                        nc.vector.tensor_copy(recipH[:], recipF[:])
                        rb_ps = psB.tile([128, 512], F32, tag="oacc", name="rb")
                        nc.tensor.matmul(rb_ps[:], ones_r[:], recipH[:],
                                         start=True, stop=True)
                        rbS = pev.tile([128, 512], FP16, tag="rbS", bufs=2)
                        nc.vector.tensor_copy(rbS[:], rb_ps[:])

                        x1lo = pev.tile([64, 512], FP16, tag="x1lo", bufs=2)
                        nc.vector.scalar_tensor_tensor(
                            x1lo[:], y_a[0:64, :], 1.0, rbS[0:64, :],
                            ALU.mult, ALU.mult,
                        )
                        nc.vector.tensor_scalar(
                            x1Tq[0:64, qt * 512:(qt + 1) * 512], x1lo[:],
                            bgn0[0:64, :], 0.0, ALU.add, ALU.max,
                        )
                        x1hn = pev.tile([64, 512], FP16, tag="x1hn", bufs=2)
                        nc.vector.scalar_tensor_tensor(
                            x1hn[:], y_b[0:64, :], 1.0, rbS[0:64, :],
                            ALU.mult, ALU.mult,
                        )
                        x1hi = pev.tile([64, 512], FP16, tag="x1hi", bufs=2)
                        nc.vector.tensor_scalar(
                            x1hi[:], x1hn[:], bgn0b[:], 0.0, ALU.add, ALU.max,
                        )
                        nc.sync.dma_start(
                            x1Tq[64:128, qt * 512:(qt + 1) * 512], x1hi[:])
                        nc.sync.dma_start(
                            aginH[qt][0:64, :],
                            x1Tq[0:64, qt * 512:(qt + 1) * 512])
                        nc.gpsimd.dma_start(aginH[qt][64:128, :], x1hi[:])
                        nc.gpsimd.collective_compute(
                            "AllGather", mybir.AluOpType.bypass,
                            ins=[aginH[qt].opt()], outs=[agoutH[qt].opt()],
                            replica_groups=[list(range(N_CORES))],
                        )
                    else:
                        # transposed adj@[h|1]: y2T[f, q] (+deg row); even/odd
                        # ping-pong accumulators avoid same-region stalls
                        y2a = psA.tile([hfp, 512], F32, tag="acc", name="y2a")
                        y2b = psA.tile([hfp, 512], F32, tag="acc", name="y2b")
                        for i in range(NJ):
                            j = jcs[i]
                            nc.tensor.matmul(
                                y2a[:] if i % 2 == 0 else y2b[:],
                                h_nat[:, j * hfp:(j + 1) * hfp],
                                slab[:, i * 512:(i + 1) * 512],
                                start=(i < 2), stop=(i >= NJ - 2),
                            )
                        yhalf = pev.tile([hfp, 512], F32, tag="yhalf", bufs=2)
                        nc.vector.tensor_copy(yhalf[:], y2a[:])
                        yev = pev.tile([hfp, 512], F32, tag="yev", bufs=2)
                        nc.vector.scalar_tensor_tensor(
                            yev[:], y2b[:], 1.0, yhalf[:], ALU.mult, ALU.add,
                        )
                        nc.sync.dma_start(
                            y_out[:, qt * 512:(qt + 1) * 512], yev[:],
                        )

    nc.compile()
    return nc


def _prep_in_maps(feat, Wgl0, bgl0, Wgnn0, bgnn0, Wgl1, bgl1, Wgnn1, bgnn1):
    s2 = np.float32(math.sqrt(2.0))
    xT = np.asarray(feat, np.float32).T

    def f32(a):
        return np.asarray(a, np.float32)

    xT16 = np.ascontiguousarray(xT.astype(np.float16))
    wglx0 = np.ascontiguousarray((f32(Wgl0) * s2).astype(np.float16))
    bglx0 = np.ascontiguousarray((f32(bgl0) * s2).reshape(-1, 1))
    wglx1 = np.ascontiguousarray((f32(Wgl1) * s2).astype(np.float16))
    bglx1 = np.ascontiguousarray((f32(bgl1) * s2).reshape(-1, 1))
    wgna0 = np.ascontiguousarray(f32(Wgnn0).astype(np.float16))
    wgna1 = np.ascontiguousarray(f32(Wgnn1).astype(np.float16))
    bgn0 = np.ascontiguousarray(f32(bgnn0).reshape(-1, 1))

    in_maps = []
    for r in range(N_CORES):
        in_maps.append({
            "xTf": xT16,
            "xTq": np.ascontiguousarray(xT16[:, r * QR:(r + 1) * QR]),
            "wglx0": wglx0, "bglx0": bglx0, "wgna0": wgna0,
            "wglx1": wglx1, "bglx1": bglx1, "wgna1": wgna1,
            "bgn0": bgn0,
        })
    return in_maps


def _postprocess(results, bgnn1):
    y = np.concatenate(
        [np.asarray(results[r]["y_out"]).T for r in range(N_CORES)], axis=0
    )  # [8192, 65]
    x2 = y[:, :F1] / y[:, F1:F1 + 1] + np.asarray(bgnn1, np.float32).reshape(1, -1)
    m = x2.max(axis=-1, keepdims=True)
    e = np.exp(x2 - m)
    return (e / e.sum(axis=-1, keepdims=True)).astype(np.float32)


def kernel(**inputs):
    from concourse.bass_utils import run_bass_kernel_spmd

    feat = np.asarray(inputs["feat_matrix"], np.float32)
    temp = float(np.asarray(inputs["temp"]))
    theta = float(np.asarray(inputs["theta"]))
    key = (round(temp, 9), round(theta, 9))
    if key not in _CACHE:
        _CACHE[key] = _build(temp, theta)
    nc = _CACHE[key]

    in_maps = _prep_in_maps(
        feat, inputs["Wgl0"], inputs["bgl0"], inputs["Wgnn0"], inputs["bgnn0"],
        inputs["Wgl1"], inputs["bgl1"], inputs["Wgnn1"], inputs["bgnn1"],
    )
    res = run_bass_kernel_spmd(nc, in_maps, list(range(N_CORES)))
    return _postprocess(res.results, inputs["bgnn1"])


# revision 21
# speedup vs baseline: 1.1932x; 1.1932x over previous
"""CDGM (graph-construction GNN) fused kernel for Trainium2, 8-way row-sharded.

Math per layer (reference):
    gl   = relu(x @ Wgl + bgl)                      [N, F]
    t_ij = ||gl_i - gl_j||^2  (via sq_i + sq_j - 2 gl_i.gl_j)
    adj  = sigmoid(-(1+temp)*sqrt(relu(t)+eps) * (t>0) + (5+theta))
    x    = (adj @ (x @ Wgnn + bgnn)) / rowsum(adj)   (+relu except last layer)
    out  = softmax(x)

Device strategy (per core r, query rows Q_r = [1024*r, 1024*(r+1))):
  - glTs = sqrt(2)*gl stored transposed [F, N] fp16; t' = -t accumulated in
    PSUM f32 as (glTs_j . glTs_q) - sq_j - sq_q (K=F fp16 matmul + K=2
    rank-2 correction for layer 0; layer 1 folds both rank-1 terms into the
    main matmul, K=66). sq comes from the same fp16-quantized glTs so the
    cancellation is consistent.
  - ACT reads the PSUM t' tiles directly: sqrt(-1*t' + DELTA) -> fp16 slab.
    The free affine (scale=-1, bias=DELTA) replaces the old DVE clamp pass;
    DELTA=0.02 keeps the sqrt domain non-negative (max positive t'
    excursion measured 0.0073) at ~1e-3 adjacency error.
  - ACT phases are strictly batched per layer (all sqrt, then all sigmoid)
    so the table set switches exactly 4 times total.
  - adj@h runs transposed on both layers: yT[f, q] += h_jc^T @ slab_jc
    (h chunks are the stationary operand). Layer 0 computes deg with
    separate ones-weight M=1 matmuls, divides via DVE (reciprocal + PE
    partition-broadcast + scalar_tensor_tensor) and writes x1T directly in
    the transposed layout the next layer wants - no PE transposes.
  - x1T is all-gathered in two 512-column chunks so the first collective
    overlaps the second half of layer-0's sigmoid/adj@h work. Layer 1
    consumes gathered columns evens-first so its setup can start as soon
    as the first chunk lands.
  - SBUF liveness: the two 32K-wide slabs (128 KB/partition) are global;
    layer-0 streams xT through double-buffered 2048-col blocks and scopes
    glTs/aug tiles in a pool that closes after the t' loop, so layer-1's
    setup allocates into space whose last readers finished by mid-layer-0
    (keeps the layer boundary pipelined). h_nat0 is global because its
    last reader is layer-0's final y matmul.
  - Final division + softmax run on host (y and deg returned raw in f32).

The harness calls kernel(**inputs) with full inputs; sharding is internal.
"""

import math

import numpy as np

N = 8192
D_IN = 256
F0, F1 = 128, 64
N_CORES = 8
QR = N // N_CORES          # query rows per core
NJ = N // 128              # 64 j-chunks of 128
SLABW = NJ * 512           # slab free width per q-tile (32768)
DELTA = 0.02               # sqrt domain guard (see module docstring)

_CACHE = {}


def _build(temp: float, theta: float):
    import concourse.bacc as bacc
    import concourse.mybir as mybir
    import concourse.tile as tile
    from concourse.tile_rust import add_dep_helper
    from contextlib import ExitStack

    DT = mybir.dt
    AF = mybir.ActivationFunctionType
    ALU = mybir.AluOpType
    F32, FP16 = DT.float32, DT.float16

    sig_scale = -(1.0 + temp)
    sig_bias = 5.0 + theta

    nc = bacc.Bacc(
        "TRN2", target_bir_lowering=False, debug=False, enable_asserts=False,
        num_devices=N_CORES,
    )

    # ---- I/O ----
    xTf_in = nc.dram_tensor("xTf", [D_IN, N], FP16, kind="ExternalInput").ap()
    xTq_in = nc.dram_tensor("xTq", [D_IN, QR], FP16, kind="ExternalInput").ap()
    wglx_in = [
        nc.dram_tensor("wglx0", [D_IN, F0], FP16, kind="ExternalInput").ap(),
        nc.dram_tensor("wglx1", [F0, F1], FP16, kind="ExternalInput").ap(),
    ]
    bglx_in = [
        nc.dram_tensor("bglx0", [F0, 1], F32, kind="ExternalInput").ap(),
        nc.dram_tensor("bglx1", [F1, 1], F32, kind="ExternalInput").ap(),
    ]
    wgna_in = [
        nc.dram_tensor("wgna0", [D_IN, F0], FP16, kind="ExternalInput").ap(),
        nc.dram_tensor("wgna1", [F0, F1], FP16, kind="ExternalInput").ap(),
    ]
    bgn0_in = nc.dram_tensor("bgn0", [F0, 1], F32, kind="ExternalInput").ap()
    y_out = nc.dram_tensor("y_out", [F1 + 1, QR], F32, kind="ExternalOutput").ap()

    with tile.TileContext(nc) as tc, ExitStack() as ctx:
        pconst = ctx.enter_context(tc.tile_pool(name="const", bufs=1))
        pouter = ctx.enter_context(tc.tile_pool(name="outer", bufs=1))
        pdram = ctx.enter_context(tc.tile_pool(name="dram", bufs=1, space="DRAM"))
        psA = ctx.enter_context(tc.tile_pool(name="psA", bufs=2, space="PSUM"))
        psB = ctx.enter_context(tc.tile_pool(name="psB", bufs=2, space="PSUM"))

        # ---- constants ----
        neghalf = pconst.tile([128, 1], FP16, tag="neghalf")
        nc.gpsimd.memset(neghalf[:], -0.5)
        ones_l = pconst.tile([128, 1], FP16, tag="ones_l")   # deg weights
        nc.gpsimd.memset(ones_l[:], 1.0)
        ones_r = pconst.tile([1, 128], FP16, tag="ones_r")   # recip broadcast
        nc.gpsimd.memset(ones_r[:], 1.0)
        sgb = pconst.tile([128, 1], F32, tag="sgb")
        nc.gpsimd.memset(sgb[:], sig_bias)
        sgs = pconst.tile([128, 1], F32, tag="sgs")
        nc.gpsimd.memset(sgs[:], sig_scale)
        delta_t = pconst.tile([128, 1], F32, tag="delta_t")
        nc.gpsimd.memset(delta_t[:], DELTA)

        fins = [D_IN, F0]
        fouts = [F0, F1]
        wgl = []
        bgl = []
        wgna = []
        for li in range(2):
            fin, fout = fins[li], fouts[li]
            nk = fin // 128
            wk = []
            ak = []
            for k in range(nk):
                t = pconst.tile([128, fout], FP16, tag=f"wgl{li}_{k}")
                nc.sync.dma_start(t[:], wglx_in[li][k * 128:(k + 1) * 128, :])
                wk.append(t)
                t = pconst.tile([128, fout], FP16, tag=f"wgna{li}_{k}")
                nc.sync.dma_start(t[:], wgna_in[li][k * 128:(k + 1) * 128, :])
                ak.append(t)
            wgl.append(wk)
            wgna.append(ak)
            bt = pconst.tile([fout, 1], F32, tag=f"bgl{li}")
            nc.sync.dma_start(bt[:], bglx_in[li][:])
            bgl.append(bt)
        bgn0 = pconst.tile([F0, 1], F32, tag="bgn0")
        nc.sync.dma_start(bgn0[:], bgn0_in[:])
        # high-half bias staged at base partition 0: the y_b accumulator
        # (features 64:127) lives on PSUM partitions 0:63, and DVE lanes
        # cannot cross partitions - the shift happens in the final DMA
        bgn0b = pconst.tile([64, 1], F32, tag="bgn0b")
        nc.sync.dma_start(bgn0b[:], bgn0_in[64:128, :])

        # layer-0 -> layer-1 bridge (x1 transposed, this core's columns),
        # all-gathered in two 512-column chunks
        x1Tq = pouter.tile([F0, QR], FP16, tag="x1tq")
        agin = pdram.tile([F0, QR], FP16, tag="agin", name="agin")
        agout = pdram.tile([N_CORES * F0, QR], FP16, tag="agout", name="agout")

        # global: the two q-half slabs and layer-0's h (its last reader is
        # the very end of layer 0, so it must never alias layer-1 tiles)
        slabs = [pouter.tile([128, SLABW], FP16, tag=f"slab{qt}", name=f"slab{qt}")
                 for qt in (0, 1)]
        h_nat0 = pouter.tile([128, NJ * (F0 + 2)], FP16, tag="hnat0")

        prev_sig = None
        for li in range(2):
            fin, fout = fins[li], fouts[li]
            nk = fin // 128
            # layer 1 fuses both -sq rank-1 terms into the main matmul as two
            # extra contraction rows (K = 66: [gl; -sqT; 1] x [gl; 1; -sqTq]).
            # Layer 0 (fout=128) cannot, and uses a separate K=2 correction.
            fka = fout + 2 if li == 1 else fout
            # h chunk layout: layer 0 [h(0:64) | ones | h(64:128) | ones]
            # (130 wide, so adj@h splits into two M=65 stationary tiles -
            # uniform M, because alternating matmul tile sizes inside open
            # PSUM accumulation groups serializes the PE pipeline at +120ns
            # per matmul; the first ones column gives deg in row 64 of y_a,
            # the second is a dummy). Layer 1: [h | ones] (65 wide).
            hfp = fout + 2 if li == 0 else fout + 1
            # layer 1's gathered x1T lands in two column-interleaved halves;
            # process 512-col chunks (and the j-chunks inside them) in
            # half-0-first order so work starts before the second AllGather.
            if li == 0:
                corder = list(range(16))
            else:
                corder = [c for c in range(16) if c % 2 == 0] + \
                         [c for c in range(16) if c % 2 == 1]
            jcs = [jc for c in corder for jc in range(4 * c, 4 * c + 4)]
            with ExitStack() as lctx:
                pmain = lctx.enter_context(tc.tile_pool(name=f"main{li}", bufs=1))
                if li == 0:
                    h_nat = h_nat0
                else:
                    h_nat = pmain.tile([128, NJ * hfp], FP16, tag="hnat")
                h3 = h_nat.rearrange("p (n f) -> p n f", f=hfp)
                if li == 0:
                    nc.gpsimd.memset(h3[:, :, 64:65], 1.0)
                    nc.gpsimd.memset(h3[:, :, 129:130], 1.0)
                else:
                    nc.gpsimd.memset(h3[:, :, fout:fout + 1], 1.0)

                with ExitStack() as tctx:
                    # closes right after the t' loop: everything here is
                    # dead once the last t' matmul has run
                    ptp = tctx.enter_context(tc.tile_pool(name=f"tp{li}", bufs=1))
                    glTs = ptp.tile([128, N], FP16, tag="glTs")
                    glTsq = ptp.tile([128, QR], FP16, tag="glTsq")
                    sqqstage = ptp.tile([1, QR], FP16, tag="sqqstage")
                    # K=128 everywhere: matmuls with K in (64,128) run at
                    # 427ns vs 216ns for K=128 (measured), so the rank-2
                    # correction tensors are zero-padded to full 128 rows.
                    if li == 0:
                        # augL: row0 = -sqT, row64 = ones, rest 0
                        # augQ: row0 = ones, row64 = -sqTq, rest 0
                        augL = ptp.tile([128, N], FP16, tag="augL")
                        augQ = ptp.tile([128, QR], FP16, tag="augQ")
                        nc.gpsimd.memset(augL[:], 0.0)
                        nc.gpsimd.memset(augL[64:65, :], 1.0)
                        nc.gpsimd.memset(augQ[:], 0.0)
                        nc.gpsimd.memset(augQ[0:1, :], 1.0)
                    else:
                        # glTs: rows 0:64 gl, row64 = ones, row65 = -sqT,
                        # rest 0; glTsq: rows 0:64 gl, row64 = -sqTq,
                        # row65 = ones, rest 0. Engines cannot start at
                        # partition 65/66, so row 65 is filled by DMA and
                        # the zero/ones fills start at partition 64.
                        nc.gpsimd.memset(glTs[64:128, :], 0.0)
                        nc.gpsimd.memset(glTs[64:65, :], 1.0)
                        nc.gpsimd.memset(glTsq[64:128, :], 0.0)
                        onesq = ptp.tile([1, QR], FP16, tag="onesq")
                        nc.gpsimd.memset(onesq[:], 1.0)
                        nc.sync.dma_start(glTsq[65:66, :], onesq[:])

                    def cstage(c, pg):
                        """evict glTs chunk c from PSUM + its -sq entries"""
                        nc.vector.tensor_scalar(
                            glTs[0:fout, c * 512:(c + 1) * 512], pg[:],
                            bgl[li][:], 0.0, ALU.add, ALU.max,
                        )
                        gl2c = pmain.tile([fout, 512], FP16, tag="gl2c")
                        nc.vector.tensor_mul(
                            gl2c[:], glTs[0:fout, c * 512:(c + 1) * 512],
                            glTs[0:fout, c * 512:(c + 1) * 512],
                        )
                        pq = psB.tile([1, 512], F32, tag="oacc")
                        nc.tensor.matmul(pq[:], neghalf[0:fout, :], gl2c[:],
                                         start=True, stop=True)
                        if li == 0:
                            nc.vector.tensor_copy(
                                augL[0:1, c * 512:(c + 1) * 512], pq[:])
                        else:
                            sqj = pmain.tile([1, 512], FP16, tag="sqj", bufs=2)
                            nc.vector.tensor_copy(sqj[:], pq[:])
                            nc.sync.dma_start(
                                glTs[65:66, c * 512:(c + 1) * 512], sqj[:])

                    # ======== setup ========
                    with ExitStack() as sctx:
                        pxt = sctx.enter_context(
                            tc.tile_pool(name=f"xt{li}", bufs=1))
                        # q side first so the first t' group unblocks early
                        if li == 0:
                            xtq = [pxt.tile([128, QR], FP16, tag=f"xtq{k}",
                                            name=f"xtq{k}") for k in range(nk)]
                            for k in range(nk):
                                nc.sync.dma_start(
                                    xtq[k][:], xTq_in[k * 128:(k + 1) * 128, :])
                        else:
                            xtq = [x1Tq]
                        for c in range(QR // 512):
                            pg = psB.tile([fout, 512], F32, tag="oacc")
                            for k in range(nk):
                                nc.tensor.matmul(
                                    pg[:], wgl[li][k][:],
                                    xtq[k][:, c * 512:(c + 1) * 512],
                                    start=(k == 0), stop=(k == nk - 1),
                                )
                            nc.vector.tensor_scalar(
                                glTsq[0:fout, c * 512:(c + 1) * 512], pg[:],
                                bgl[li][:], 0.0, ALU.add, ALU.max,
                            )
                            gl2c = pmain.tile([fout, 512], FP16, tag="gl2c")
                            nc.vector.tensor_mul(
                                gl2c[:], glTsq[0:fout, c * 512:(c + 1) * 512],
                                glTsq[0:fout, c * 512:(c + 1) * 512],
                            )
                            pq = psB.tile([1, 512], F32, tag="oacc")
                            nc.tensor.matmul(pq[:], neghalf[0:fout, :], gl2c[:],
                                             start=True, stop=True)
                            nc.vector.tensor_copy(
                                sqqstage[0:1, c * 512:(c + 1) * 512], pq[:])
                        if li == 0:
                            nc.sync.dma_start(augQ[64:65, :], sqqstage[:])
                        else:
                            nc.sync.dma_start(glTsq[64:65, :], sqqstage[:])

                        if li == 0:
                            # stream xT through double-buffered 1024-col
                            # blocks; build glTs, -sq and h_nat per block
                            for b in range(8):
                                xb = [pxt.tile([128, 1024], FP16, tag=f"xtf{k}",
                                               name=f"xtf{k}", bufs=3)
                                      for k in range(nk)]
                                for k in range(nk):
                                    nc.sync.dma_start(
                                        xb[k][:],
                                        xTf_in[k * 128:(k + 1) * 128,
                                               b * 1024:(b + 1) * 1024],
                                    )
                                for c4 in range(2):
                                    c = 2 * b + c4
                                    pg = psB.tile([fout, 512], F32, tag="oacc")
                                    for k in range(nk):
                                        nc.tensor.matmul(
                                            pg[:], wgl[li][k][:],
                                            xb[k][:, c4 * 512:(c4 + 1) * 512],
                                            start=(k == 0), stop=(k == nk - 1),
                                        )
                                    cstage(c, pg)
                                for g4 in range(2):
                                    j0 = 8 * b + 4 * g4
                                    ph = psB.tile([128, 4 * fout], F32, tag="oacc")
                                    for t in range(4):
                                        jl = 4 * g4 + t
                                        sl = ph[:, t * fout:(t + 1) * fout]
                                        for k in range(nk):
                                            nc.tensor.matmul(
                                                sl, xb[k][:, jl * 128:(jl + 1) * 128],
                                                wgna[li][k][:],
                                                start=(k == 0), stop=(k == nk - 1),
                                            )
                                    ph3 = ph[:].rearrange(
                                        "p (n f) -> p n f", f=fout)
                                    nc.vector.tensor_copy(
                                        h3[:, j0:j0 + 4, 0:64], ph3[:, :, 0:64])
                                    nc.vector.tensor_copy(
                                        h3[:, j0:j0 + 4, 65:129],
                                        ph3[:, :, 64:128])
                        else:
                            # gathered x1T; DMAs split across two queues
                            xtf = pmain.tile([128, N], FP16, tag="xtf1f")
                            xtf1h = xtf
                            for r in range(N_CORES):
                                eng = nc.sync if r % 2 == 0 else nc.gpsimd
                                eng.dma_start(
                                    xtf[:, r * QR:(r + 1) * QR],
                                    agout[r * F0:(r + 1) * F0, :],
                                )
                            for c in corder:
                                pg = psB.tile([fout, 512], F32, tag="oacc")
                                nc.tensor.matmul(
                                    pg[:], wgl[li][0][:],
                                    xtf[:, c * 512:(c + 1) * 512],
                                    start=True, stop=True,
                                )
                                cstage(c, pg)

                    # ======== t' + sqrt (ACT consumes PSUM directly) ========
                    # qt0/qt1 interleaved: consecutive matmuls then stream
                    # DIFFERENT rhs slices, which keeps the LDWEIGHTS
                    # pull-ahead working (a repeated rhs with changing
                    # weights measures 334-427 ns/MM vs 216 warm)
                    sqrt_insts = []
                    for g0 in range(0, NJ, 2):
                        tms = [psA.tile([128, 1024], F32, tag="tmac",
                                        name=f"tm{qt}") for qt in (0, 1)]
                        for t in range(2):
                            j = jcs[g0 + t]
                            for qt in (0, 1):
                                nc.tensor.matmul(
                                    tms[qt][:, t * 512:(t + 1) * 512],
                                    glTs[:, j * 128:(j + 1) * 128],
                                    glTsq[:, qt * 512:(qt + 1) * 512],
                                    start=True, stop=(li == 1),
                                )
                        if li == 0:
                            for t in range(2):
                                j = jcs[g0 + t]
                                for qt in (0, 1):
                                    nc.tensor.matmul(
                                        tms[qt][:, t * 512:(t + 1) * 512],
                                        augL[:, j * 128:(j + 1) * 128],
                                        augQ[:, qt * 512:(qt + 1) * 512],
                                        start=False, stop=True,
                                    )
                        for qt in (0, 1):
                            si = nc.scalar.activation(
                                slabs[qt][:, g0 * 512:(g0 + 2) * 512],
                                tms[qt][:, 0:1024],
                                AF.Sqrt, bias=delta_t[:], scale=-1.0,
                            )
                            if prev_sig is not None:
                                add_dep_helper(si.ins, prev_sig.ins, sync=False,
                                               reason="act-table phase batching")
                            sqrt_insts.append(si)

                if li == 1:
                    # h_nat built after the t' matmuls so it never delays
                    # the first sqrt of this layer (it is only needed by
                    # the adj@h phase)
                    for g0 in range(0, NJ, 4):
                        js = jcs[g0:g0 + 4]
                        ph = psB.tile([128, 4 * fout], F32, tag="oacc")
                        for t in range(4):
                            n_ = js[t]
                            nc.tensor.matmul(
                                ph[:, t * fout:(t + 1) * fout],
                                xtf1h[:, n_ * 128:(n_ + 1) * 128],
                                wgna[li][0][:],
                                start=True, stop=True,
                            )
                        nc.vector.tensor_copy(
                            h3[:, js[0]:js[0] + 4, 0:fout],
                            ph[:].rearrange("p (n f) -> p n f", f=fout),
                        )

                # ======== sigmoid + adj@h + bridge ========
                sig_insts = []
                for qt in (0, 1):
                    slab = slabs[qt]
                    for g in range(4):
                        sl = slab[:, g * 8192:(g + 1) * 8192]
                        si = nc.scalar.activation(sl, sl, AF.Sigmoid,
                                                  bias=sgb[:], scale=sgs[:])
                        # Keep all sigmoids after every sqrt so the ACT
                        # table set switches exactly once per phase.
                        add_dep_helper(si.ins, sqrt_insts[-1].ins, sync=False,
                                       reason="act-table phase batching")
                        sig_insts.append(si)
                prev_sig = sig_insts[-1]

                # created here (not at layer start) so its space is carved out
                # of what the tp pool freed, after the t' loop
                pev = lctx.enter_context(tc.tile_pool(name=f"ev{li}", bufs=1))
                wt = psB.tile([128, 512], F32, tag="oacc", name="warm")
                for w in range(16):
                    mi = nc.tensor.matmul(wt[:], wgna[0][0][:],
                                          h_nat0[:, 0:512], start=True,
                                          stop=True)
                    if w == 0:
                        add_dep_helper(mi.ins, sig_insts[0].ins, sync=False,
                                       reason="HAM warm-up gate")
                for qt in (0, 1):
                    slab = slabs[qt]
                    if li == 0:
                        # adj@[h|1] split into M=65 / M=64 halves: two
                        # accumulators alternate, so consecutive matmuls never
                        # accumulate into the same PSUM region (which would
                        # serialize on the PE drain); deg rides along as
                        # row 64 of y_a via the ones column of h.
                        y_a = psA.tile([65, 512], F32, tag="acc", name="y_a")
                        y_b = psA.tile([65, 512], F32, tag="acc", name="y_b")
                        # staggered by one slab slice so consecutive
                        # matmuls stream different rhs regions (see t' note)
                        for i in range(NJ + 1):
                            if i < NJ:
                                j = jcs[i]
                                nc.tensor.matmul(
                                    y_a[:], h_nat[:, j * hfp:j * hfp + 65],
                                    slab[:, i * 512:(i + 1) * 512],
                                    start=(i == 0), stop=(i == NJ - 1),
                                )
                            if i > 0:
                                j = jcs[i - 1]
                                nc.tensor.matmul(
                                    y_b[:],
                                    h_nat[:, j * hfp + 65:(j + 1) * hfp],
                                    slab[:, (i - 1) * 512:i * 512],
                                    start=(i == 1), stop=(i == NJ),
                                )
                        degS = pev.tile([1, 512], F32, tag="degS")
                        nc.vector.tensor_copy(degS[:], y_a[64:65, :])
                        recipF = pev.tile([1, 512], F32, tag="recipF")
                        nc.vector.reciprocal_approx_fast(recipF[:], degS[:])
                        recipH = pev.tile([1, 512], FP16, tag="recipH")
                        nc.vector.tensor_copy(recipH[:], recipF[:])
                        rb_ps = psB.tile([128, 512], F32, tag="oacc", name="rb")
                        nc.tensor.matmul(rb_ps[:], ones_r[:], recipH[:],
                                         start=True, stop=True)
                        rbS = pev.tile([128, 512], FP16, tag="rbS", bufs=2)
                        nc.vector.tensor_copy(rbS[:], rb_ps[:])

                        x1lo = pev.tile([64, 512], FP16, tag="x1lo", bufs=2)
                        nc.vector.scalar_tensor_tensor(
                            x1lo[:], y_a[0:64, :], 1.0, rbS[0:64, :],
                            ALU.mult, ALU.mult,
                        )
                        nc.vector.tensor_scalar(
                            x1Tq[0:64, qt * 512:(qt + 1) * 512], x1lo[:],
                            bgn0[0:64, :], 0.0, ALU.add, ALU.max,
                        )
                        x1hn = pev.tile([64, 512], FP16, tag="x1hn", bufs=2)
                        nc.vector.scalar_tensor_tensor(
                            x1hn[:], y_b[0:64, :], 1.0, rbS[0:64, :],
                            ALU.mult, ALU.mult,
                        )
                        x1hi = pev.tile([64, 512], FP16, tag="x1hi", bufs=2)
                        nc.vector.tensor_scalar(
                            x1hi[:], x1hn[:], bgn0b[:], 0.0, ALU.add, ALU.max,
                        )
                        nc.sync.dma_start(
                            x1Tq[64:128, qt * 512:(qt + 1) * 512], x1hi[:])
                        nc.sync.dma_start(
                            agin[0:64, qt * 512:(qt + 1) * 512],
                            x1Tq[0:64, qt * 512:(qt + 1) * 512])
                        nc.gpsimd.dma_start(
                            agin[64:128, qt * 512:(qt + 1) * 512], x1hi[:])
                        if qt == 1:
                            nc.gpsimd.collective_compute(
                                "AllGather", mybir.AluOpType.bypass,
                                ins=[agin.opt()], outs=[agout.opt()],
                                replica_groups=[list(range(N_CORES))],
                            )
                    else:
                        # transposed adj@[h|1]: y2T[f, q] (+deg row); even/odd
                        # ping-pong accumulators avoid same-region stalls
                        y2a = psA.tile([hfp, 512], F32, tag="acc", name="y2a")
                        y2b = psA.tile([hfp, 512], F32, tag="acc", name="y2b")
                        for i in range(NJ):
                            j = jcs[i]
                            nc.tensor.matmul(
                                y2a[:] if i % 2 == 0 else y2b[:],
                                h_nat[:, j * hfp:(j + 1) * hfp],
                                slab[:, i * 512:(i + 1) * 512],
                                start=(i < 2), stop=(i >= NJ - 2),
                            )
                        yhalf = pev.tile([hfp, 512], F32, tag="yhalf", bufs=2)
                        nc.vector.tensor_copy(yhalf[:], y2a[:])
                        yev = pev.tile([hfp, 512], F32, tag="yev", bufs=2)
                        nc.vector.scalar_tensor_tensor(
                            yev[:], y2b[:], 1.0, yhalf[:], ALU.mult, ALU.add,
                        )
                        nc.sync.dma_start(
                            y_out[:, qt * 512:(qt + 1) * 512], yev[:],
                        )

    nc.compile()
    return nc


def _prep_in_maps(feat, Wgl0, bgl0, Wgnn0, bgnn0, Wgl1, bgl1, Wgnn1, bgnn1):
    s2 = np.float32(math.sqrt(2.0))
    xT = np.asarray(feat, np.float32).T

    def f32(a):
        return np.asarray(a, np.float32)

    xT16 = np.ascontiguousarray(xT.astype(np.float16))
    wglx0 = np.ascontiguousarray((f32(Wgl0) * s2).astype(np.float16))
    bglx0 = np.ascontiguousarray((f32(bgl0) * s2).reshape(-1, 1))
    wglx1 = np.ascontiguousarray((f32(Wgl1) * s2).astype(np.float16))
    bglx1 = np.ascontiguousarray((f32(bgl1) * s2).reshape(-1, 1))
    wgna0 = np.ascontiguousarray(f32(Wgnn0).astype(np.float16))
    wgna1 = np.ascontiguousarray(f32(Wgnn1).astype(np.float16))
    bgn0 = np.ascontiguousarray(f32(bgnn0).reshape(-1, 1))

    in_maps = []
    for r in range(N_CORES):
        in_maps.append({
            "xTf": xT16,
            "xTq": np.ascontiguousarray(xT16[:, r * QR:(r + 1) * QR]),
            "wglx0": wglx0, "bglx0": bglx0, "wgna0": wgna0,
            "wglx1": wglx1, "bglx1": bglx1, "wgna1": wgna1,
            "bgn0": bgn0,
        })
    return in_maps


def _postprocess(results, bgnn1):
    y = np.concatenate(
        [np.asarray(results[r]["y_out"]).T for r in range(N_CORES)], axis=0
    )  # [8192, 65]
    x2 = y[:, :F1] / y[:, F1:F1 + 1] + np.asarray(bgnn1, np.float32).reshape(1, -1)
    m = x2.max(axis=-1, keepdims=True)
    e = np.exp(x2 - m)
    return (e / e.sum(axis=-1, keepdims=True)).astype(np.float32)


def kernel(**inputs):
    from concourse.bass_utils import run_bass_kernel_spmd

    feat = np.asarray(inputs["feat_matrix"], np.float32)
    temp = float(np.asarray(inputs["temp"]))
    theta = float(np.asarray(inputs["theta"]))
    key = (round(temp, 9), round(theta, 9))
    if key not in _CACHE:
        _CACHE[key] = _build(temp, theta)
    nc = _CACHE[key]

    in_maps = _prep_in_maps(
        feat, inputs["Wgl0"], inputs["bgl0"], inputs["Wgnn0"], inputs["bgnn0"],
        inputs["Wgl1"], inputs["bgl1"], inputs["Wgnn1"], inputs["bgnn1"],
    )
    res = run_bass_kernel_spmd(nc, in_maps, list(range(N_CORES)))
    return _postprocess(res.results, inputs["bgnn1"])
